# revision 26
# baseline (speedup 1.0000x reference)
"""TRN2 Bass kernel for AttentionBlock3D (GroupNorm + MHA + proj + residual).

Shapes (hardcoded): x [1, 512, 4, 32, 32] -> x2d [C=512, N=4096],
qkv_w [1536, 512], proj_w [512, 512], 8 heads x head_dim 64, GN groups 32.

Distribution: one head per NeuronCore (8 cores, tensor parallel).
Per core h:
  - GroupNorm is folded into the qkv GEMM: per-channel scale s_c and shift
    t_c are computed on-device from bn_stats of bf16 x; the head's qkv
    weights are scaled by s_c and the shift folds into the bias via a
    small matvec (b' = b + W @ t).
  - q/k are produced DUPLICATED across both 64-partition halves (weight
    columns repeated on host) so S^T = k^T q (contraction dim 64) can be
    row-packed into both halves of the PE array (2 concurrent matmuls).
  - Flash-style attention: S^T tiles in PSUM -> ScalarE exp (scale=1/8
    folded in, no max-subtraction: |logits| < 1.5) -> P bf16 in SBUF ->
    PV accumulation with a ones-column appended to v^T producing the
    softmax denominators in row 64 of the PSUM accumulator.
  - Per-core head output O_h [64, 4096] is AllGathered (bf16), then each
    core computes its 64-row output-channel slice of the projection and
    adds the residual x + proj_b (preadded on host, fp32).
Host gathers the 8 [64, 4096] fp32 slices and stacks them.
"""

import sys

for _p in ("/opt/trn_rl_repo", "/root/.axon_site/_ro/trn_rl_repo"):
    if _p not in sys.path:
        sys.path.insert(0, _p)

import numpy as np
import ml_dtypes

import concourse.bass as bass
import concourse.bacc as bacc
import concourse.mybir as mybir
from concourse import tile
from concourse.bass_utils import run_bass_kernel_spmd

BF16 = ml_dtypes.bfloat16
FP32 = mybir.dt.float32
BF = mybir.dt.bfloat16

C = 512          # channels
N = 4096         # sequence (4*32*32)
NH = 8           # heads
HD = 64          # head dim
G = 32           # groupnorm groups
EPS = 1e-5
JT = C // 128    # 4 channel tiles
NB = N // 512    # 8 column blocks of 512
NS = N // 128    # 32 s-subtiles of 128
SCALE = HD ** -0.5  # 0.125

# s-subtiles per PSUM S-tile (free dim of one exp op = 512*edge)
S_GROUPS = [3, 3, 3, 3, 3, 3, 3, 3, 3, 3, 2]
assert sum(S_GROUPS) == NS

_CACHED = {}
DEBUG = False


def _build_program():
    nc = bacc.Bacc(
        "TRN2", target_bir_lowering=False, debug=False, num_devices=NH
    )

    # ---------------- kernel I/O ----------------
    xb_h = nc.declare_dram_parameter("xb", [C, N], BF, isOutput=False)
    xr_h = nc.declare_dram_parameter("xr", [HD, N], FP32, isOutput=False)
    wqkvT_h = nc.declare_dram_parameter("wqkvT", [C, 320], BF, isOutput=False)
    bqkv_h = nc.declare_dram_parameter("bqkv", [128, 3], FP32, isOutput=False)
    gnw_h = nc.declare_dram_parameter("gnw", [128, JT], FP32, isOutput=False)
    gnb_h = nc.declare_dram_parameter("gnb", [128, JT], FP32, isOutput=False)
    sel_h = nc.declare_dram_parameter("sel", [128, 8], FP32, isOutput=False)
    selT_h = nc.declare_dram_parameter("selT", [8, 128], FP32, isOutput=False)
    ident_h = nc.declare_dram_parameter("ident", [HD, HD], BF, isOutput=False)
    wpT_h = nc.declare_dram_parameter("wpT", [C, HD], BF, isOutput=False)
    out_h = nc.declare_dram_parameter("out", [HD, N], FP32, isOutput=True)
    if DEBUG:
        dbg = {
            "d_spp": ([128, JT], FP32),
            "d_tpp": ([128, JT], FP32),
            "d_me": ([128, JT, 2], FP32),
            "d_bq": ([128, 1], FP32),
            "d_bv": ([HD, 1], FP32),
            "d_q2": ([128, N], BF),
            "d_k2": ([128, N], BF),
            "d_v": ([HD, N], BF),
            "d_vt": ([128, NS, HD + 1], BF),
            "d_o": ([HD, N], BF),
            "d_oa": ([128, JT, N], BF),
            "d_p": ([128, 1536], BF),
            "d_dsb": ([NB, 128, 1024], FP32),
            "d_pvs": ([NB, HD, 512], FP32),
        }
        dbg_h = {
            k: nc.declare_dram_parameter(k, list(sh), dt, isOutput=True)
            for k, (sh, dt) in dbg.items()
        }

    AF = mybir.ActivationFunctionType
    ALU = mybir.AluOpType

    with tile.TileContext(nc) as tc:
        with (
            tc.tile_pool(name="const", bufs=1) as cpool,
            tc.tile_pool(name="big", bufs=1) as big,
            tc.tile_pool(name="work", bufs=2) as work,
            tc.tile_pool(name="ppool", bufs=3) as ppool,
            tc.tile_pool(name="dram", bufs=1, space="DRAM") as dram,
        ):
            # ---------------- load constants / inputs ----------------
            XB = big.tile([128, JT, N], BF, tag="xb")
            for j in range(JT):
                nc.sync.dma_start(
                    XB[:, j, :],
                    xb_h[:].rearrange("(j p) n -> j p n", p=128)[j],
                )
            WT = cpool.tile([128, JT, 320], BF, tag="wt")
            nc.sync.dma_start(
                WT[:], wqkvT_h[:].rearrange("(j p) m -> p j m", p=128)
            )
            WPT = cpool.tile([128, JT, HD], BF, tag="wpt")
            nc.sync.dma_start(
                WPT[:], wpT_h[:].rearrange("(j p) m -> p j m", p=128)
            )
            bqkv_t = cpool.tile([128, 3], FP32, tag="bqkv")
            nc.sync.dma_start(bqkv_t[:], bqkv_h[:])
            gnw_t = cpool.tile([128, JT], FP32, tag="gnw")
            nc.sync.dma_start(gnw_t[:], gnw_h[:])
            gnb_t = cpool.tile([128, JT], FP32, tag="gnb")
            nc.sync.dma_start(gnb_t[:], gnb_h[:])
            sel_t = cpool.tile([128, 8], FP32, tag="sel")
            nc.sync.dma_start(sel_t[:], sel_h[:])
            selT_t = cpool.tile([8, 128], FP32, tag="selt")
            nc.sync.dma_start(selT_t[:], selT_h[:])
            ident_t = cpool.tile([HD, HD], BF, tag="ident")
            nc.sync.dma_start(ident_t[:], ident_h[:])
            XR = big.tile([HD, N], FP32, tag="xr")
            nc.sync.dma_start(XR[:], xr_h[:])
            ones_t = cpool.tile([128, 64], FP32, tag="ones")
            nc.gpsimd.memset(ones_t[:], 1.0)
            ones_bf = cpool.tile([128, 64], BF, tag="onesbf")
            nc.gpsimd.memset(ones_bf[:], 1.0)

            # ---------------- GroupNorm stats (pipelined per channel-tile) ----
            # per-channel mean/var via bn_stats/bn_aggr, group-aggregated
            # across partitions with a one-hot selector matmul; each tile j's
            # scaled weights become ready as soon as ITS stats land, so the
            # qkv GEMM k-tiles can start without a global stats barrier.
            ME = cpool.tile([128, JT, 2], FP32, tag="me")  # (mean_c, E[x^2]_c)
            s_pp = cpool.tile([128, JT], FP32, tag="spp")
            t_pp = cpool.tile([128, JT], FP32, tag="tpp")
            t_bf = cpool.tile([128, JT], BF, tag="tbf")
            WTs = cpool.tile([128, JT, 320], BF, tag="wts")
            rm = cpool.tile([8, JT, 2], FP32, tag="rm")
            with tc.tile_pool(name="stps", bufs=1, space="PSUM") as stps:
                gexp = stps.tile([128, JT, 2], FP32, tag="gexp")
                for j in range(JT):
                    st6 = work.tile([128, 8, 6], FP32, tag="st6")
                    for b in range(8):
                        nc.vector.bn_stats(
                            st6[:, b, :], XB[:, j, b * 512 : (b + 1) * 512]
                        )
                    st2 = work.tile([128, 2], FP32, tag="st2")
                    nc.vector.bn_aggr(st2[:], st6[:])
                    # ME[:,j,0] = mean_c ; ME[:,j,1] = var_c + mean_c^2
                    nc.vector.tensor_copy(ME[:, j, 0:1], st2[:, 0:1])
                    tmp = work.tile([128, 1], FP32, tag="sttmp")
                    nc.vector.tensor_tensor(tmp[:], st2[:, 0:1], st2[:, 0:1], ALU.mult)
                    nc.vector.tensor_tensor(ME[:, j, 1:2], st2[:, 1:2], tmp[:], ALU.add)
                    g = stps.tile([8, 2], FP32, tag=f"gps{j}")
                    nc.tensor.matmul(g[:], sel_t[:], ME[:, j, :], start=True, stop=True)
                    # rm[:, j, 0] = rsqrt(var_g + eps), rm[:, j, 1] = mean_g
                    gsb = work.tile([8, 2], FP32, tag="gsb")
                    nc.vector.tensor_copy(gsb[:], g[:])
                    nc.vector.tensor_scalar_mul(rm[:, j, 1:2], gsb[:, 0:1], 1.0 / 16.0)
                    eg = work.tile([8, 1], FP32, tag="eg")
                    nc.vector.tensor_scalar_mul(eg[:], gsb[:, 1:2], 1.0 / 16.0)
                    mg2 = work.tile([8, 1], FP32, tag="mg2")
                    nc.vector.tensor_tensor(mg2[:], rm[:, j, 1:2], rm[:, j, 1:2], ALU.mult)
                    vg = work.tile([8, 1], FP32, tag="vg")
                    nc.vector.tensor_tensor(vg[:], eg[:], mg2[:], ALU.subtract)
                    nc.vector.tensor_scalar_add(vg[:], vg[:], float(EPS))
                    # rsqrt = exp(-0.5*ln(var+eps)) — same ACT table set as Exp
                    lnv = work.tile([8, 1], FP32, tag="lnv")
                    nc.scalar.activation(lnv[:], vg[:], AF.Ln)
                    nc.scalar.activation(rm[:, j, 0:1], lnv[:], AF.Exp, scale=-0.5)
                    # expand groups -> channels via selT matmul
                    nc.tensor.matmul(
                        gexp[:, j, :], selT_t[:], rm[:, j, :], start=True, stop=True
                    )
                    rsq_j = work.tile([128, 1], FP32, tag="rsqj")
                    nc.vector.tensor_copy(rsq_j[:], gexp[:, j, 0:1])
                    # s_c = gn_w * rsqrt ; t_c = gn_b - mean * s_c
                    nc.vector.tensor_tensor(
                        s_pp[:, j : j + 1], gnw_t[:, j : j + 1], rsq_j[:], ALU.mult
                    )
                    nc.vector.tensor_tensor(
                        t_pp[:, j : j + 1], gexp[:, j, 1:2], s_pp[:, j : j + 1], ALU.mult
                    )
                    nc.vector.tensor_tensor(
                        t_pp[:, j : j + 1], gnb_t[:, j : j + 1], t_pp[:, j : j + 1],
                        ALU.subtract,
                    )
                    nc.vector.tensor_copy(t_bf[:, j : j + 1], t_pp[:, j : j + 1])
                    # scaled weights W' = W^T * s_c (per-partition scale)
                    nc.vector.tensor_scalar_mul(
                        WTs[:, j, :], WT[:, j, :], s_pp[:, j : j + 1]
                    )

            # fused bias b' = qkv_b + W @ t   (unscaled W)
            bq_sb = cpool.tile([128, 1], FP32, tag="bqsb")
            bk_sb = cpool.tile([128, 1], FP32, tag="bksb")
            bv_sb = cpool.tile([HD, 1], FP32, tag="bvsb")
            with tc.tile_pool(name="bps", bufs=1, space="PSUM") as bps:
                bq_ps = bps.tile([128, 1], FP32, tag="bq")
                bk_ps = bps.tile([128, 1], FP32, tag="bk")
                bv_ps = bps.tile([HD, 1], FP32, tag="bv")
                for j in range(JT):
                    st, sp = j == 0, j == JT - 1
                    nc.tensor.matmul(bq_ps[:], WT[:, j, 0:128], t_bf[:, j : j + 1], start=st, stop=sp)
                    nc.tensor.matmul(bk_ps[:], WT[:, j, 128:256], t_bf[:, j : j + 1], start=st, stop=sp)
                    nc.tensor.matmul(bv_ps[:], WT[:, j, 256:320], t_bf[:, j : j + 1], start=st, stop=sp)
                nc.vector.tensor_tensor(bq_sb[:], bq_ps[:], bqkv_t[:, 0:1], ALU.add)
                nc.vector.tensor_tensor(bk_sb[:], bk_ps[:], bqkv_t[:, 1:2], ALU.add)
                nc.vector.tensor_tensor(bv_sb[:], bv_ps[:], bqkv_t[0:HD, 2:3], ALU.add)

            # ---------------- qkv GEMM ----------------
            Q2 = big.tile([128, N], BF, tag="q2")   # q duplicated in both halves
            K2 = big.tile([128, N], BF, tag="k2")   # k duplicated in both halves
            V = big.tile([HD, N], BF, tag="v")

            def qkv_block(pool, nb, what):
                ns = slice(nb * 512, (nb + 1) * 512)
                if what == "k":
                    ps = pool.tile([128, 512], FP32, tag="psk", bufs=2)
                    cols, dst, bias = slice(128, 256), K2, bk_sb
                elif what == "q":
                    ps = pool.tile([128, 512], FP32, tag="psq", bufs=2)
                    cols, dst, bias = slice(0, 128), Q2, bq_sb
                else:
                    ps = pool.tile([HD, 512], FP32, tag="psv", bufs=2)
                    cols, dst, bias = slice(256, 320), V, bv_sb
                for j in range(JT):
                    nc.tensor.matmul(
                        ps[:], WTs[:, j, cols], XB[:, j, ns],
                        start=(j == 0), stop=(j == JT - 1),
                    )
                nc.vector.tensor_scalar_add(dst[:, ns], ps[:], bias[:])

            with tc.tile_pool(name="kvps", bufs=1, space="PSUM") as kvps:
                for nb in range(NB):
                    qkv_block(kvps, nb, "k")
                for nb in range(NB):
                    qkv_block(kvps, nb, "v")

            # v^T (32 PE transposes of [64,128] -> [128,64]), ones col 64
            VT = big.tile([128, NS, HD + 1], BF, tag="vt")
            nc.gpsimd.memset(VT[:], 1.0)
            with (
                tc.tile_pool(name="trps", bufs=4, space="PSUM") as trps,
                tc.tile_pool(name="qps", bufs=1, space="PSUM") as qps,
            ):
                for st in range(NS):
                    tr = trps.tile([128, HD], BF, tag="tr")
                    nc.tensor.transpose(
                        tr[:], V[:, st * 128 : (st + 1) * 128], ident_t[:]
                    )
                    nc.vector.tensor_copy(VT[:, st, 0:HD], tr[:])
                for nb in range(NB):
                    qkv_block(qps, nb, "q")

            # ---------------- attention ----------------
            O = big.tile([HD, N], BF, tag="o")
            cc_ins, cc_outs = [], []
            for ch in range(4):
                cc_in = dram.tile([HD, N // 4], BF, tag=f"ccin{ch}")
                cc_out = dram.tile(
                    [C, N // 4], BF, tag=f"ccout{ch}", addr_space="Shared"
                )
                cc_ins.append(cc_in)
                cc_outs.append(cc_out)

            with tc.tile_pool(name="attps", bufs=1, space="PSUM") as attps:

                def normalize(tb, pv):
                    # softmax normalize: r = 1/denom, broadcast via K=1 matmul.
                    # Emitted AFTER the next t-block's s-loop so the PE-stream
                    # rd-matmul doesn't stall on the DVE reciprocal latency.
                    ts = slice(tb * 512, (tb + 1) * 512)
                    dsb = work.tile([128, 1024], FP32, tag="dsb", name="dsb")
                    nc.vector.tensor_copy(dsb[64:65, 0:512], pv[HD : HD + 1, :])
                    nc.vector.reciprocal(dsb[64:65, 512:1024], dsb[64:65, 0:512])
                    rbf = work.tile([128, 512], BF, tag="rbf", name="rbf")
                    nc.vector.tensor_copy(rbf[64:65, :], dsb[64:65, 512:1024])
                    # rd shares the "s" tag slots so pv can double-buffer
                    rd_t = attps.tile([128, 1536], FP32, tag="s", bufs=2, name="rd_t")
                    rd = rd_t[0:HD, 0:512]
                    nc.tensor.matmul(
                        rd, ones_bf[64:65, 0:HD], rbf[64:65, :],
                        start=True, stop=True,
                    )
                    pvs = work.tile([HD, 512], FP32, tag="pvs", name="pvs")
                    nc.vector.tensor_copy(pvs[:], pv[0:HD, :])
                    nc.vector.tensor_tensor(O[:, ts], pvs[:], rd, ALU.mult)
                    if DEBUG:
                        nc.sync.dma_start(dbg_h["d_dsb"][tb], dsb[:])
                        nc.sync.dma_start(dbg_h["d_pvs"][tb], pvs[:])
                    # stream the AllGather out in quarters as t-blocks finish
                    if tb % 2 == 1:
                        ch = tb // 2
                        cs = slice(ch * 1024, (ch + 1) * 1024)
                        nc.sync.dma_start(cc_ins[ch][:], O[:, cs])
                        nc.gpsimd.collective_compute(
                            "AllGather",
                            ALU.bypass,
                            replica_groups=[list(range(NH))],
                            ins=[cc_ins[ch].opt()],
                            outs=[cc_outs[ch].opt()],
                        )

                def pv_group(pv, P, gs, gsz):
                    for u in range(gsz):
                        g = gs + u
                        nc.tensor.matmul(
                            pv[:], VT[:, g, :], P[:, u * 512 : (u + 1) * 512],
                            start=(g == 0), stop=(g == NS - 1),
                        )

                pending = None
                prev = None  # PV runs one exp-group behind, across tb bounds
                for tb in range(NB):
                    ts = slice(tb * 512, (tb + 1) * 512)
                    pv = attps.tile([HD + 1, 512], FP32, tag="pv", bufs=2)
                    gs = 0
                    for gsz in S_GROUPS:
                        fd = gsz * 512
                        S = attps.tile([128, 1536], FP32, tag="s", bufs=2)
                        P = ppool.tile([128, 1536], BF, tag="p")
                        for u in range(gsz):
                            g = gs + u
                            h0 = 64 * (g % 2)
                            nc.tensor.matmul(
                                S[:, u * 512 : (u + 1) * 512],
                                K2[h0 : h0 + 64, g * 128 : (g + 1) * 128],
                                Q2[h0 : h0 + 64, ts],
                                start=True, stop=True,
                            )
                        nc.scalar.activation(
                            P[:, 0:fd], S[:, 0:fd], AF.Exp, scale=float(SCALE)
                        )
                        if DEBUG and tb == 0 and gs == 0:
                            nc.sync.dma_start(dbg_h["d_p"][:], P[:])
                        if prev is not None:
                            pv_group(*prev)
                        prev = (pv, P, gs, gsz)
                        gs += gsz
                        if gs == 9 and pending is not None:
                            # previous block's normalize, deep enough into
                            # this block's s-loop that the recip has finished
                            normalize(*pending)
                            pending = None

                    pending = (tb, pv)
                pv_group(*prev)
                normalize(*pending)

            # ---------------- projection slices per AG chunk ----------------
            OUT = big.tile([HD, N], FP32, tag="outsb")
            OAs = []
            for ch in range(4):
                oa = big.tile([128, JT, N // 4], BF, tag=f"oa{ch}", name=f"oa{ch}")
                nc.sync.dma_start(
                    oa[:], cc_outs[ch][:].rearrange("(j p) n -> p j n", p=128)
                )
                OAs.append(oa)

            with tc.tile_pool(name="prps", bufs=2, space="PSUM") as prps:
                for nb in range(NB):
                    ns = slice(nb * 512, (nb + 1) * 512)
                    oa = OAs[nb // 2]
                    os_ = slice((nb % 2) * 512, (nb % 2) * 512 + 512)
                    pp = prps.tile([HD, 512], FP32, tag="pp")
                    for j in range(JT):
                        nc.tensor.matmul(
                            pp[:], WPT[:, j, :], oa[:, j, os_],
                            start=(j == 0), stop=(j == JT - 1),
                        )
                    nc.vector.tensor_tensor(OUT[:, ns], pp[:], XR[:, ns], ALU.add)
                    nc.sync.dma_start(out_h[:, ns], OUT[:, ns])

            if DEBUG:
                nc.sync.dma_start(dbg_h["d_spp"][:], s_pp[:])
                nc.sync.dma_start(dbg_h["d_tpp"][:], t_pp[:])
                nc.sync.dma_start(dbg_h["d_me"][:], ME[:])
                nc.sync.dma_start(dbg_h["d_bq"][:], bq_sb[:])
                nc.sync.dma_start(dbg_h["d_bv"][:], bv_sb[:])
                nc.sync.dma_start(dbg_h["d_q2"][:], Q2[:])
                nc.sync.dma_start(dbg_h["d_k2"][:], K2[:])
                nc.sync.dma_start(dbg_h["d_v"][:], V[:])
                nc.sync.dma_start(dbg_h["d_vt"][:], VT[:])
                nc.sync.dma_start(dbg_h["d_o"][:], O[:])
                nc.sync.dma_start(dbg_h["d_oa"][:], OA[:])

    nc.compile()
    return nc


def _prep_inputs(x, gn_w, gn_b, qkv_w, qkv_b, proj_w, proj_b):
    x2 = np.ascontiguousarray(np.asarray(x, np.float32).reshape(C, N))
    gn_w = np.asarray(gn_w, np.float32)
    gn_b = np.asarray(gn_b, np.float32)
    qkv_w = np.asarray(qkv_w, np.float32)
    qkv_b = np.asarray(qkv_b, np.float32)
    proj_w = np.asarray(proj_w, np.float32)
    proj_b = np.asarray(proj_b, np.float32)

    xb = x2.astype(BF16)
    gnw_pp = np.ascontiguousarray(gn_w.reshape(JT, 128).T)
    gnb_pp = np.ascontiguousarray(gn_b.reshape(JT, 128).T)
    sel = np.zeros((128, 8), np.float32)
    sel[np.arange(128), np.arange(128) // 16] = 1.0
    selT = np.ascontiguousarray(sel.T)
    ident = np.eye(HD, dtype=BF16)

    in_maps = []
    for h in range(NH):
        r = slice(h * HD, (h + 1) * HD)
        Wq = qkv_w[h * HD : (h + 1) * HD]
        Wk = qkv_w[C + h * HD : C + (h + 1) * HD]
        Wv = qkv_w[2 * C + h * HD : 2 * C + (h + 1) * HD]
        wqkvT = np.concatenate(
            [Wq.T, Wq.T, Wk.T, Wk.T, Wv.T], axis=1
        ).astype(BF16)  # [512, 320]
        bqkv = np.zeros((128, 3), np.float32)
        bqkv[:, 0] = np.tile(qkv_b[h * HD : (h + 1) * HD], 2)
        bqkv[:, 1] = np.tile(qkv_b[C + h * HD : C + (h + 1) * HD], 2)
        bqkv[:HD, 2] = qkv_b[2 * C + h * HD : 2 * C + (h + 1) * HD]
        wpT = np.ascontiguousarray(proj_w[r, :].T).astype(BF16)
        xr = x2[r, :] + proj_b[r, None]
        in_maps.append(
            {
                "xb": xb,
                "xr": np.ascontiguousarray(xr),
                "wqkvT": np.ascontiguousarray(wqkvT),
                "bqkv": bqkv,
                "gnw": gnw_pp,
                "gnb": gnb_pp,
                "sel": sel,
                "selT": selT,
                "ident": ident,
                "wpT": wpT,
            }
        )
    return in_maps


def run(inputs_maps, trace=False, **kwargs):
    if "nc" not in _CACHED:
        _CACHED["nc"] = _build_program()
    return run_bass_kernel_spmd(
        _CACHED["nc"], inputs_maps, core_ids=list(range(NH)), trace=trace, **kwargs
    )


def kernel(x, gn_w, gn_b, qkv_w, qkv_b, proj_w, proj_b):
    in_maps = _prep_inputs(x, gn_w, gn_b, qkv_w, qkv_b, proj_w, proj_b)
    res = run(in_maps)
    rows = [np.asarray(res.results[h]["out"], np.float32) for h in range(NH)]
    out = np.concatenate(rows, axis=0)
    return out.reshape(np.asarray(x).shape)


if __name__ == "__main__":
    nc = _build_program()
    print("program built OK")
